# revision 21
# baseline (speedup 1.0000x reference)
"""Chamfer loss kernel for Trainium2 (8 NeuronCores, SPMD data-parallel over batch).

Math: for each batch b, d2[n, m] = |p_n|^2 + |g_m|^2 - 2 p_n.g_m is produced
directly by an augmented matmul on the PE. To run the PE at bf16 rate (1
cycle/row instead of fp32's 4) without losing fp32 accuracy, every fp32
operand is split into three bf16 terms (h + m + l); retaining the product
pairs hh, hm, mh, hl, lh, mm reproduces each fp32 product to ~2^-27 rel.
With 3 coords x 6 pairs + 3 |p|^2 rows + 3 |g|^2 rows the contraction dim
is K=24, all bf16, accumulated exactly into fp32 PSUM.

Per PSUM megatile [128, 1024] (2 banks): ScalarE copies d2 to SBUF (freeing
the PSUM bank early and keeping both DVE reads off PSUM); DVE takes the
free-axis row min and the running column-state min; PE transposes the final
column state so DVE can finish the column mins. All DVE fp32 ops run at 1x
(~1.1 elem/cycle/lane measured) — two passes over every d2 element is the
hard floor, and DVE sits at ~97% busy.
sqrt + means run on the host (min/max commute with sqrt/clamp).

Each core handles 4 of the 32 batches. No collectives; host combines scalars.
"""

import sys

for _p in ("/opt/trn_rl_repo",):
    if _p not in sys.path:
        sys.path.insert(0, _p)

from contextlib import ExitStack
from functools import lru_cache

import ml_dtypes
import numpy as np

import concourse.bass as bass
import concourse.tile as tile
from concourse import bacc, masks, mybir
from concourse.bass_utils import run_bass_kernel_spmd

F32 = mybir.dt.float32
BF16 = mybir.dt.bfloat16
MIN = mybir.AluOpType.min
NPBF16 = ml_dtypes.bfloat16

B, N, M = 32, 4096, 4096
NCORES = 8
BPC = B // NCORES  # batches per core
K = 24             # augmented contraction dim (3 coords x 6 bf16 pairs + 2x3 norm rows)
PCH = 128          # pred chunk size (PE partitions)
NP = N // PCH      # 32 pred chunks
FD = 1024          # psum tile free size (2 PSUM banks)
FDV = 1024         # DVE working chunk (psum tiles staged into SBUF by ScalarE)
NG = M // FDV      # gt chunks per batch row pass
MMN = 512          # matmul moving free dim (one fp32 PSUM bank)
BIG = 3.0e38
STAGE_SBUF = True  # stage d2 into SBUF via ScalarE before the DVE passes


def _build_program():
    nc = bacc.Bacc(
        "TRN2", target_bir_lowering=False, debug=False, num_devices=NCORES
    )
    lhs = nc.dram_tensor("lhs", [BPC * K, N], BF16, kind="ExternalInput").ap()
    rhs = nc.dram_tensor("rhs", [BPC * K, M], BF16, kind="ExternalInput").ap()
    rowmin = nc.dram_tensor("rowmin", [BPC * PCH, NP], F32, kind="ExternalOutput").ap()
    colmin = nc.dram_tensor(
        "colmin", [BPC * PCH, M // PCH], F32, kind="ExternalOutput"
    ).ap()

    with tile.TileContext(nc) as tc, ExitStack() as ctx:
        const_pool = ctx.enter_context(tc.tile_pool(name="const", bufs=1))
        ident = const_pool.tile([PCH, PCH], F32)
        masks.make_identity(nc, ident[:])
        inf_t = const_pool.tile([PCH, FDV], F32)
        nc.gpsimd.memset(inf_t[:], BIG)

        lr_pool = ctx.enter_context(tc.tile_pool(name="lr", bufs=2))
        col_pool = ctx.enter_context(tc.tile_pool(name="col", bufs=NG + 1))
        d2_pool = ctx.enter_context(tc.tile_pool(name="d2", bufs=4))
        acc_pool = ctx.enter_context(tc.tile_pool(name="acc", bufs=2))
        scr_pool = ctx.enter_context(tc.tile_pool(name="scr", bufs=8))
        psum_pool = ctx.enter_context(tc.tile_pool(name="psum", bufs=3, space="PSUM"))
        psumt_pool = ctx.enter_context(tc.tile_pool(name="psumt", bufs=1, space="PSUM"))

        for i in range(BPC):
            L = lr_pool.tile([K, N], BF16, tag="L")
            nc.sync.dma_start(L[:], lhs[K * i : K * (i + 1), :])
            R = lr_pool.tile([K, M], BF16, tag="R")
            nc.sync.dma_start(R[:], rhs[K * i : K * (i + 1), :])

            colstate = [
                col_pool.tile([PCH, FDV], F32, tag="cs", name=f"cs_{i}_{g}")
                for g in range(NG)
            ]
            rowacc = acc_pool.tile([PCH, NP], F32, tag="rowacc")
            colfin = acc_pool.tile([PCH, M // PCH], F32, tag="colfin")

            rowpart = scr_pool.tile(
                [PCH, NP * NG], F32, tag="rowpart", name=f"rp_{i}"
            )
            for p in range(NP):
                for g in range(NG):
                    # stage a [PCH, FDV] block of d2 into SBUF via ScalarE so
                    # the two DVE passes below run from SBUF; or feed PSUM
                    # directly when staging is disabled
                    d2 = None
                    if STAGE_SBUF:
                        d2 = d2_pool.tile([PCH, FDV], F32, tag="d2")
                    for half in range(FDV // FD):
                        ps = psum_pool.tile([PCH, FD], F32, tag="ps")
                        base = FDV * g + FD * half
                        for s in range(FD // MMN):
                            nc.tensor.matmul(
                                ps[:, MMN * s : MMN * (s + 1)],
                                lhsT=L[:, PCH * p : PCH * (p + 1)],
                                rhs=R[:, base + MMN * s : base + MMN * (s + 1)],
                                start=True,
                                stop=True,
                            )
                        if STAGE_SBUF:
                            nc.scalar.copy(d2[:, FD * half : FD * (half + 1)], ps[:])
                        else:
                            d2 = ps
                    # row (pred-point) partial min over this gt chunk
                    nc.vector.tensor_reduce(
                        out=rowpart[:, p * NG + g : p * NG + g + 1],
                        in_=d2[:],
                        axis=mybir.AxisListType.X,
                        op=MIN,
                    )
                    # colstate accumulation (min over pred chunks).
                    src0 = inf_t[:] if p == 0 else colstate[g][:]
                    nc.vector.tensor_tensor(
                        out=colstate[g][:], in0=src0, in1=d2[:], op=MIN
                    )
            # fold all NP x NG partials into the final row mins in one op
            nc.vector.tensor_reduce(
                out=rowacc[:],
                in_=rowpart[:].rearrange("p (a b) -> p a b", b=NG),
                axis=mybir.AxisListType.X,
                op=MIN,
            )

            # Column mins: partition-reduce the colstate tiles via PE
            # transposes batched into a shared PSUM strip, then one 3D
            # min-reduce per strip of 8 transposed blocks.
            NBLK = FD // PCH  # 8 transposed blocks per psumt strip
            for g in range(NG):
                for grp in range(FDV // FD):
                    pt = psumt_pool.tile([PCH, FD], F32, tag="pt")
                    for blk in range(NBLK):
                        nc.tensor.transpose(
                            pt[:, PCH * blk : PCH * (blk + 1)],
                            colstate[g][
                                :, FD * grp + PCH * blk : FD * grp + PCH * (blk + 1)
                            ],
                            ident[:],
                        )
                    j0 = g * (FDV // PCH) + grp * NBLK
                    nc.vector.tensor_reduce(
                        out=colfin[:, j0 : j0 + NBLK],
                        in_=pt[:].rearrange("p (b f) -> p b f", f=PCH),
                        axis=mybir.AxisListType.X,
                        op=MIN,
                    )

            nc.sync.dma_start(rowmin[PCH * i : PCH * (i + 1), :], rowacc[:])
            nc.sync.dma_start(colmin[PCH * i : PCH * (i + 1), :], colfin[:])

    nc.compile()
    return nc


@lru_cache(maxsize=1)
def _get_program():
    return _build_program()


def _split3(x):
    """fp32 -> three bf16 terms whose sum matches x to ~2^-27 rel."""
    h = x.astype(NPBF16)
    r = x - h.astype(np.float32)
    m = r.astype(NPBF16)
    l = (r - m.astype(np.float32)).astype(NPBF16)
    return h, m, l


def _make_inputs(pred, gt):
    """Host-side packing of the K=24 bf16 split operands, per core."""
    pred = np.ascontiguousarray(pred, dtype=np.float32)
    gt = np.ascontiguousarray(gt, dtype=np.float32)
    p2 = np.einsum("bnd,bnd->bn", pred, pred)
    g2 = np.einsum("bmd,bmd->bm", gt, gt)
    Lr, Rr = [], []
    for d in range(3):
        u = np.float32(-2.0) * pred[:, :, d]
        v = gt[:, :, d]
        uh, um, ul = _split3(u)
        vh, vm, vl = _split3(v)
        # product pairs kept: hh, hm, mh, hl, lh, mm
        Lr += [uh, uh, um, uh, ul, um]
        Rr += [vh, vm, vh, vl, vh, vm]
    ph, pm, pl = _split3(p2)
    gh, gm, gl = _split3(g2)
    ones_n = np.ones_like(p2, dtype=NPBF16)
    ones_m = np.ones_like(g2, dtype=NPBF16)
    Lr += [ph, pm, pl, ones_n, ones_n, ones_n]
    Rr += [ones_m, ones_m, ones_m, gh, gm, gl]
    lhs = np.stack(Lr, axis=1)  # [B, K, N] bf16
    rhs = np.stack(Rr, axis=1)  # [B, K, M] bf16
    in_maps = []
    for c in range(NCORES):
        sl = slice(c * BPC, (c + 1) * BPC)
        in_maps.append(
            {
                "lhs": np.ascontiguousarray(lhs[sl].reshape(BPC * K, N)),
                "rhs": np.ascontiguousarray(rhs[sl].reshape(BPC * K, M)),
            }
        )
    return in_maps


def _finish(results):
    rowmins = np.stack([r["rowmin"] for r in results])  # [8, BPC*128, 32]
    colmins = np.stack([r["colmin"] for r in results])
    ch2 = np.sqrt(np.maximum(rowmins.astype(np.float64), 1e-12)).mean()
    ch1 = np.sqrt(np.maximum(colmins.astype(np.float64), 1e-12)).mean()
    return np.asarray(ch1 + ch2, dtype=np.float32)


def kernel(pred, gt):
    nc = _get_program()
    in_maps = _make_inputs(pred, gt)
    res = run_bass_kernel_spmd(nc, in_maps, list(range(NCORES)))
    return _finish(res.results)


if __name__ == "__main__":
    rng = np.random.default_rng(0)
    pred = rng.standard_normal((B, N, 3), dtype=np.float32)
    gt = rng.standard_normal((B, M, 3), dtype=np.float32)
    print(kernel(pred, gt))


# revision 22
# speedup vs baseline: 1.0033x; 1.0033x over previous
"""Chamfer loss kernel for Trainium2 (8 NeuronCores, SPMD data-parallel over batch).

Math: the device computes s = -d2 where d2[n, m] = |p_n|^2 + |g_m|^2 - 2 p_n.g_m,
so every min the loss needs becomes a max on device (GpSimd's
partition_all_reduce only offers max). s is produced directly by an augmented
matmul on the PE. To run the PE at bf16 rate (1 cycle/row instead of fp32's 4)
without losing fp32 accuracy, every fp32 operand is split into three bf16
terms (h + m + l); retaining the product pairs hh, hm, mh, hl, lh, mm
reproduces each fp32 product to ~2^-27 rel. With 3 coords x 6 pairs + 3 |p|^2
rows + 3 |g|^2 rows the contraction dim is K=24, all bf16, accumulated
exactly into fp32 PSUM.

Per PSUM megatile [128, 1024] (2 banks): ScalarE copies s to SBUF (freeing
the PSUM bank early and keeping both DVE reads off PSUM); DVE takes the
free-axis row max and the running column-state max; GpSimd finishes the
column maxes with partition_all_reduce(max). All DVE fp32 ops run at 1x
(~1.1 elem/cycle/lane measured) — two passes over every element is the hard
floor, and DVE sits at ~97% busy. sqrt + means run on the host (min/max
commute with sqrt/clamp after negation).

Each core handles 4 of the 32 batches. No collectives; host combines scalars.
"""

import sys

for _p in ("/opt/trn_rl_repo",):
    if _p not in sys.path:
        sys.path.insert(0, _p)

from contextlib import ExitStack
from functools import lru_cache

import ml_dtypes
import numpy as np

import concourse.bass as bass
import concourse.tile as tile
from concourse import bacc, bass_isa, mybir
from concourse.bass_utils import run_bass_kernel_spmd

F32 = mybir.dt.float32
BF16 = mybir.dt.bfloat16
MAX = mybir.AluOpType.max
NPBF16 = ml_dtypes.bfloat16

B, N, M = 32, 4096, 4096
NCORES = 8
BPC = B // NCORES  # batches per core
K = 24             # augmented contraction dim (3 coords x 6 bf16 pairs + 2x3 norm rows)
PCH = 128          # pred chunk size (PE partitions)
NP = N // PCH      # 32 pred chunks
FD = 1024          # psum tile free size (2 PSUM banks)
FDV = 1024         # DVE working chunk (psum tiles staged into SBUF by ScalarE)
NG = M // FDV      # gt chunks per batch row pass
MMN = 512          # matmul moving free dim (one fp32 PSUM bank)
BIG = 3.0e38


def _build_program():
    nc = bacc.Bacc(
        "TRN2", target_bir_lowering=False, debug=False, num_devices=NCORES
    )
    lhs = nc.dram_tensor("lhs", [BPC * K, N], BF16, kind="ExternalInput").ap()
    rhs = nc.dram_tensor("rhs", [BPC * K, M], BF16, kind="ExternalInput").ap()
    rowmin = nc.dram_tensor("rowmin", [BPC * PCH, NP], F32, kind="ExternalOutput").ap()
    colmin = nc.dram_tensor("colmin", [BPC, M], F32, kind="ExternalOutput").ap()

    with tile.TileContext(nc) as tc, ExitStack() as ctx:
        const_pool = ctx.enter_context(tc.tile_pool(name="const", bufs=1))
        neg_t = const_pool.tile([PCH, FDV], F32)
        nc.gpsimd.memset(neg_t[:], -BIG)

        lr_pool = ctx.enter_context(tc.tile_pool(name="lr", bufs=2))
        col_pool = ctx.enter_context(tc.tile_pool(name="col", bufs=NG + 1))
        red_pool = ctx.enter_context(tc.tile_pool(name="red", bufs=2))
        d2_pool = ctx.enter_context(tc.tile_pool(name="d2", bufs=4))
        acc_pool = ctx.enter_context(tc.tile_pool(name="acc", bufs=2))
        scr_pool = ctx.enter_context(tc.tile_pool(name="scr", bufs=8))
        psum_pool = ctx.enter_context(tc.tile_pool(name="psum", bufs=4, space="PSUM"))

        for i in range(BPC):
            L = lr_pool.tile([K, N], BF16, tag="L")
            nc.sync.dma_start(L[:], lhs[K * i : K * (i + 1), :])
            R = lr_pool.tile([K, M], BF16, tag="R")
            nc.sync.dma_start(R[:], rhs[K * i : K * (i + 1), :])

            colstate = [
                col_pool.tile([PCH, FDV], F32, tag="cs", name=f"cs_{i}_{g}")
                for g in range(NG)
            ]
            rowacc = acc_pool.tile([PCH, NP], F32, tag="rowacc")

            rowpart = scr_pool.tile(
                [PCH, NP * NG], F32, tag="rowpart", name=f"rp_{i}"
            )
            for p in range(NP):
                for g in range(NG):
                    # stage a [PCH, FDV] block of s = -d2 into SBUF via
                    # ScalarE so both DVE passes below run from SBUF
                    d2 = d2_pool.tile([PCH, FDV], F32, tag="d2")
                    for half in range(FDV // FD):
                        ps = psum_pool.tile([PCH, FD], F32, tag="ps")
                        base = FDV * g + FD * half
                        for s in range(FD // MMN):
                            nc.tensor.matmul(
                                ps[:, MMN * s : MMN * (s + 1)],
                                lhsT=L[:, PCH * p : PCH * (p + 1)],
                                rhs=R[:, base + MMN * s : base + MMN * (s + 1)],
                                start=True,
                                stop=True,
                            )
                        nc.scalar.copy(d2[:, FD * half : FD * (half + 1)], ps[:])
                    # row (pred-point) partial max over this gt chunk
                    nc.vector.tensor_reduce(
                        out=rowpart[:, p * NG + g : p * NG + g + 1],
                        in_=d2[:],
                        axis=mybir.AxisListType.X,
                        op=MAX,
                    )
                    # colstate accumulation (max over pred chunks).
                    src0 = neg_t[:] if p == 0 else colstate[g][:]
                    nc.vector.tensor_tensor(
                        out=colstate[g][:], in0=src0, in1=d2[:], op=MAX
                    )
            # fold all NP x NG partials into the final row maxes in one op
            nc.vector.tensor_reduce(
                out=rowacc[:],
                in_=rowpart[:].rearrange("p (a b) -> p a b", b=NG),
                axis=mybir.AxisListType.X,
                op=MAX,
            )

            # Column maxes: partition all-reduce on GpSimd, then ship row 0.
            for g in range(NG):
                csr = red_pool.tile([PCH, FDV], F32, tag="csr", name=f"csr_{i}_{g}")
                nc.gpsimd.partition_all_reduce(
                    csr[:], colstate[g][:], channels=PCH,
                    reduce_op=bass_isa.ReduceOp.max,
                )
                nc.sync.dma_start(
                    colmin[i : i + 1, FDV * g : FDV * (g + 1)], csr[0:1, :]
                )

            nc.sync.dma_start(rowmin[PCH * i : PCH * (i + 1), :], rowacc[:])

    nc.compile()
    return nc


@lru_cache(maxsize=1)
def _get_program():
    return _build_program()


def _split3(x):
    """fp32 -> three bf16 terms whose sum matches x to ~2^-27 rel."""
    h = x.astype(NPBF16)
    r = x - h.astype(np.float32)
    m = r.astype(NPBF16)
    l = (r - m.astype(np.float32)).astype(NPBF16)
    return h, m, l


def _make_inputs(pred, gt):
    """Host-side packing of the K=24 bf16 split operands (for -d2), per core."""
    pred = np.ascontiguousarray(pred, dtype=np.float32)
    gt = np.ascontiguousarray(gt, dtype=np.float32)
    p2 = np.einsum("bnd,bnd->bn", pred, pred)
    g2 = np.einsum("bmd,bmd->bm", gt, gt)
    Lr, Rr = [], []
    for d in range(3):
        u = np.float32(2.0) * pred[:, :, d]  # +2 so the dot yields -d2
        v = gt[:, :, d]
        uh, um, ul = _split3(u)
        vh, vm, vl = _split3(v)
        # product pairs kept: hh, hm, mh, hl, lh, mm
        Lr += [uh, uh, um, uh, ul, um]
        Rr += [vh, vm, vh, vl, vh, vm]
    ph, pm, pl = _split3(-p2)
    gh, gm, gl = _split3(g2)
    ones_n = np.ones_like(p2, dtype=NPBF16)
    neg_n = -ones_n
    ones_m = np.ones_like(g2, dtype=NPBF16)
    Lr += [ph, pm, pl, neg_n, neg_n, neg_n]
    Rr += [ones_m, ones_m, ones_m, gh, gm, gl]
    lhs = np.stack(Lr, axis=1)  # [B, K, N] bf16
    rhs = np.stack(Rr, axis=1)  # [B, K, M] bf16
    in_maps = []
    for c in range(NCORES):
        sl = slice(c * BPC, (c + 1) * BPC)
        in_maps.append(
            {
                "lhs": np.ascontiguousarray(lhs[sl].reshape(BPC * K, N)),
                "rhs": np.ascontiguousarray(rhs[sl].reshape(BPC * K, M)),
            }
        )
    return in_maps


def _finish(results):
    # device values are maxes of -d2: negate back to d2 mins
    rowmins = -np.stack([r["rowmin"] for r in results])  # [8, BPC*128, 32]
    colmins = -np.stack([r["colmin"] for r in results])  # [8, BPC, M]
    ch2 = np.sqrt(np.maximum(rowmins.astype(np.float64), 1e-12)).mean()
    ch1 = np.sqrt(np.maximum(colmins.astype(np.float64), 1e-12)).mean()
    return np.asarray(ch1 + ch2, dtype=np.float32)


def kernel(pred, gt):
    nc = _get_program()
    in_maps = _make_inputs(pred, gt)
    res = run_bass_kernel_spmd(nc, in_maps, list(range(NCORES)))
    return _finish(res.results)


if __name__ == "__main__":
    rng = np.random.default_rng(0)
    pred = rng.standard_normal((B, N, 3), dtype=np.float32)
    gt = rng.standard_normal((B, M, 3), dtype=np.float32)
    print(kernel(pred, gt))


# revision 23
# speedup vs baseline: 1.0304x; 1.0269x over previous
"""Chamfer loss kernel for Trainium2 (8 NeuronCores, SPMD data-parallel over batch).

Math: the device computes s = -d2 where d2[n, m] = |p_n|^2 + |g_m|^2 - 2 p_n.g_m,
so every min the loss needs becomes a max on device (GpSimd's
partition_all_reduce only offers max). s is produced directly by an augmented
matmul on the PE. To run the PE at bf16 rate (1 cycle/row instead of fp32's 4)
without losing fp32 accuracy, every fp32 operand is split into three bf16
terms (h + m + l); retaining the product pairs hh, hm, mh, hl, lh, mm
reproduces each fp32 product to ~2^-27 rel. With 3 coords x 6 pairs + 3 |p|^2
rows + 3 |g|^2 rows the contraction dim is K=24, all bf16, accumulated
exactly into fp32 PSUM.

Per PSUM megatile [128, 1024] (2 banks): ScalarE copies s to SBUF (freeing
the PSUM bank early and keeping both DVE reads off PSUM); DVE takes the
free-axis row max and the running column-state max; GpSimd finishes the
column maxes with partition_all_reduce(max). All DVE fp32 ops run at 1x
(~1.1 elem/cycle/lane measured) — two passes over every element is the hard
floor, and DVE sits at ~97% busy. sqrt + means run on the host (min/max
commute with sqrt/clamp after negation).

Each core handles 4 of the 32 batches. No collectives; host combines scalars.
"""

import sys

for _p in ("/opt/trn_rl_repo",):
    if _p not in sys.path:
        sys.path.insert(0, _p)

from contextlib import ExitStack
from functools import lru_cache

import ml_dtypes
import numpy as np

import concourse.bass as bass
import concourse.tile as tile
from concourse import bacc, bass_isa, mybir
from concourse.bass_utils import run_bass_kernel_spmd

F32 = mybir.dt.float32
BF16 = mybir.dt.bfloat16
MAX = mybir.AluOpType.max
NPBF16 = ml_dtypes.bfloat16

B, N, M = 32, 4096, 4096
NCORES = 8
BPC = B // NCORES  # batches per core
K = 24             # augmented contraction dim (3 coords x 6 bf16 pairs + 2x3 norm rows)
PCH = 128          # pred chunk size (PE partitions)
NP = N // PCH      # 32 pred chunks
FD = 2048          # psum tile free size (4 PSUM banks)
FDV = 2048         # DVE working chunk (psum tiles staged into SBUF by ScalarE)
NG = M // FDV      # gt chunks per batch row pass
MMN = 512          # matmul moving free dim (one fp32 PSUM bank)
BIG = 3.0e38


def _build_program():
    nc = bacc.Bacc(
        "TRN2", target_bir_lowering=False, debug=False, num_devices=NCORES
    )
    lhs = nc.dram_tensor("lhs", [BPC * K, N], BF16, kind="ExternalInput").ap()
    rhs = nc.dram_tensor("rhs", [BPC * K, M], BF16, kind="ExternalInput").ap()
    rowmin = nc.dram_tensor("rowmin", [BPC * PCH, NP], F32, kind="ExternalOutput").ap()
    colmin = nc.dram_tensor("colmin", [BPC, M], F32, kind="ExternalOutput").ap()

    with tile.TileContext(nc) as tc, ExitStack() as ctx:
        const_pool = ctx.enter_context(tc.tile_pool(name="const", bufs=1))
        neg_t = const_pool.tile([PCH, FDV], F32)
        nc.gpsimd.memset(neg_t[:], -BIG)

        lr_pool = ctx.enter_context(tc.tile_pool(name="lr", bufs=2))
        col_pool = ctx.enter_context(tc.tile_pool(name="col", bufs=NG + 1))
        red_pool = ctx.enter_context(tc.tile_pool(name="red", bufs=2))
        d2_pool = ctx.enter_context(tc.tile_pool(name="d2", bufs=4))
        acc_pool = ctx.enter_context(tc.tile_pool(name="acc", bufs=2))
        scr_pool = ctx.enter_context(tc.tile_pool(name="scr", bufs=8))
        psum_pool = ctx.enter_context(tc.tile_pool(name="psum", bufs=2, space="PSUM"))

        for i in range(BPC):
            L = lr_pool.tile([K, N], BF16, tag="L")
            nc.sync.dma_start(L[:], lhs[K * i : K * (i + 1), :])
            R = lr_pool.tile([K, M], BF16, tag="R")
            nc.sync.dma_start(R[:], rhs[K * i : K * (i + 1), :])

            colstate = [
                col_pool.tile([PCH, FDV], F32, tag="cs", name=f"cs_{i}_{g}")
                for g in range(NG)
            ]
            rowacc = acc_pool.tile([PCH, NP], F32, tag="rowacc")

            rowpart = scr_pool.tile(
                [PCH, NP * NG], F32, tag="rowpart", name=f"rp_{i}"
            )
            for p in range(NP):
                for g in range(NG):
                    # stage a [PCH, FDV] block of s = -d2 into SBUF via
                    # ScalarE so both DVE passes below run from SBUF
                    d2 = d2_pool.tile([PCH, FDV], F32, tag="d2")
                    for half in range(FDV // FD):
                        ps = psum_pool.tile([PCH, FD], F32, tag="ps")
                        base = FDV * g + FD * half
                        for s in range(FD // MMN):
                            nc.tensor.matmul(
                                ps[:, MMN * s : MMN * (s + 1)],
                                lhsT=L[:, PCH * p : PCH * (p + 1)],
                                rhs=R[:, base + MMN * s : base + MMN * (s + 1)],
                                start=True,
                                stop=True,
                            )
                        nc.scalar.copy(d2[:, FD * half : FD * (half + 1)], ps[:])
                    # row (pred-point) partial max over this gt chunk
                    nc.vector.tensor_reduce(
                        out=rowpart[:, p * NG + g : p * NG + g + 1],
                        in_=d2[:],
                        axis=mybir.AxisListType.X,
                        op=MAX,
                    )
                    # colstate accumulation (max over pred chunks).
                    src0 = neg_t[:] if p == 0 else colstate[g][:]
                    nc.vector.tensor_tensor(
                        out=colstate[g][:], in0=src0, in1=d2[:], op=MAX
                    )
            # fold all NP x NG partials into the final row maxes in one op
            nc.vector.tensor_reduce(
                out=rowacc[:],
                in_=rowpart[:].rearrange("p (a b) -> p a b", b=NG),
                axis=mybir.AxisListType.X,
                op=MAX,
            )

            # Column maxes: partition all-reduce on GpSimd, then ship row 0.
            for g in range(NG):
                csr = red_pool.tile([PCH, FDV], F32, tag="csr", name=f"csr_{i}_{g}")
                nc.gpsimd.partition_all_reduce(
                    csr[:], colstate[g][:], channels=PCH,
                    reduce_op=bass_isa.ReduceOp.max,
                )
                nc.sync.dma_start(
                    colmin[i : i + 1, FDV * g : FDV * (g + 1)], csr[0:1, :]
                )

            nc.sync.dma_start(rowmin[PCH * i : PCH * (i + 1), :], rowacc[:])

    nc.compile()
    return nc


@lru_cache(maxsize=1)
def _get_program():
    return _build_program()


def _split3(x):
    """fp32 -> three bf16 terms whose sum matches x to ~2^-27 rel."""
    h = x.astype(NPBF16)
    r = x - h.astype(np.float32)
    m = r.astype(NPBF16)
    l = (r - m.astype(np.float32)).astype(NPBF16)
    return h, m, l


def _make_inputs(pred, gt):
    """Host-side packing of the K=24 bf16 split operands (for -d2), per core."""
    pred = np.ascontiguousarray(pred, dtype=np.float32)
    gt = np.ascontiguousarray(gt, dtype=np.float32)
    p2 = np.einsum("bnd,bnd->bn", pred, pred)
    g2 = np.einsum("bmd,bmd->bm", gt, gt)
    Lr, Rr = [], []
    for d in range(3):
        u = np.float32(2.0) * pred[:, :, d]  # +2 so the dot yields -d2
        v = gt[:, :, d]
        uh, um, ul = _split3(u)
        vh, vm, vl = _split3(v)
        # product pairs kept: hh, hm, mh, hl, lh, mm
        Lr += [uh, uh, um, uh, ul, um]
        Rr += [vh, vm, vh, vl, vh, vm]
    ph, pm, pl = _split3(-p2)
    gh, gm, gl = _split3(g2)
    ones_n = np.ones_like(p2, dtype=NPBF16)
    neg_n = -ones_n
    ones_m = np.ones_like(g2, dtype=NPBF16)
    Lr += [ph, pm, pl, neg_n, neg_n, neg_n]
    Rr += [ones_m, ones_m, ones_m, gh, gm, gl]
    lhs = np.stack(Lr, axis=1)  # [B, K, N] bf16
    rhs = np.stack(Rr, axis=1)  # [B, K, M] bf16
    in_maps = []
    for c in range(NCORES):
        sl = slice(c * BPC, (c + 1) * BPC)
        in_maps.append(
            {
                "lhs": np.ascontiguousarray(lhs[sl].reshape(BPC * K, N)),
                "rhs": np.ascontiguousarray(rhs[sl].reshape(BPC * K, M)),
            }
        )
    return in_maps


def _finish(results):
    # device values are maxes of -d2: negate back to d2 mins
    rowmins = -np.stack([r["rowmin"] for r in results])  # [8, BPC*128, 32]
    colmins = -np.stack([r["colmin"] for r in results])  # [8, BPC, M]
    ch2 = np.sqrt(np.maximum(rowmins.astype(np.float64), 1e-12)).mean()
    ch1 = np.sqrt(np.maximum(colmins.astype(np.float64), 1e-12)).mean()
    return np.asarray(ch1 + ch2, dtype=np.float32)


def kernel(pred, gt):
    nc = _get_program()
    in_maps = _make_inputs(pred, gt)
    res = run_bass_kernel_spmd(nc, in_maps, list(range(NCORES)))
    return _finish(res.results)


if __name__ == "__main__":
    rng = np.random.default_rng(0)
    pred = rng.standard_normal((B, N, 3), dtype=np.float32)
    gt = rng.standard_normal((B, M, 3), dtype=np.float32)
    print(kernel(pred, gt))


# revision 25
# speedup vs baseline: 1.0314x; 1.0010x over previous
"""Chamfer loss kernel for Trainium2 (8 NeuronCores, SPMD data-parallel over batch).

Math: the device computes s = -d2 where d2[n, m] = |p_n|^2 + |g_m|^2 - 2 p_n.g_m,
so every min the loss needs becomes a max on device (GpSimd's
partition_all_reduce only offers max). s is produced directly by an augmented
matmul on the PE. To run the PE at bf16 rate (1 cycle/row instead of fp32's 4)
without losing fp32 accuracy, every fp32 operand is split into three bf16
terms (h + m + l); retaining the product pairs hh, hm, mh, hl, lh, mm
reproduces each fp32 product to ~2^-27 rel. With 3 coords x 6 pairs + 3 |p|^2
rows + 3 |g|^2 rows the contraction dim is K=24, all bf16, accumulated
exactly into fp32 PSUM.

Per PSUM megatile [128, 1024] (2 banks): ScalarE copies s to SBUF (freeing
the PSUM bank early and keeping both DVE reads off PSUM); DVE takes the
free-axis row max and the running column-state max; GpSimd finishes the
column maxes with partition_all_reduce(max). All DVE fp32 ops run at 1x
(~1.1 elem/cycle/lane measured) — two passes over every element is the hard
floor, and DVE sits at ~97% busy. sqrt + means run on the host (min/max
commute with sqrt/clamp after negation).

Each core handles 4 of the 32 batches. No collectives; host combines scalars.
"""

import sys

for _p in ("/opt/trn_rl_repo",):
    if _p not in sys.path:
        sys.path.insert(0, _p)

from contextlib import ExitStack
from functools import lru_cache

import ml_dtypes
import numpy as np

import concourse.bass as bass
import concourse.tile as tile
from concourse import bacc, bass_isa, mybir
from concourse.bass_utils import run_bass_kernel_spmd

F32 = mybir.dt.float32
BF16 = mybir.dt.bfloat16
MAX = mybir.AluOpType.max
NPBF16 = ml_dtypes.bfloat16

B, N, M = 32, 4096, 4096
NCORES = 8
BPC = B // NCORES  # batches per core
K = 24             # augmented contraction dim (3 coords x 6 bf16 pairs + 2x3 norm rows)
PCH = 128          # pred chunk size (PE partitions)
NP = N // PCH      # 32 pred chunks
FD = 2048          # psum tile free size (4 PSUM banks)
FDV = 2048         # DVE working chunk (psum tiles staged into SBUF by ScalarE)
NG = M // FDV      # gt chunks per batch row pass
MMN = 512          # matmul moving free dim (one fp32 PSUM bank)
BIG = 3.0e38


def _build_program():
    nc = bacc.Bacc(
        "TRN2", target_bir_lowering=False, debug=False, num_devices=NCORES
    )
    lhs = nc.dram_tensor("lhs", [BPC * K, N], BF16, kind="ExternalInput").ap()
    rhs = nc.dram_tensor("rhs", [BPC * K, M], BF16, kind="ExternalInput").ap()
    rowmin = nc.dram_tensor("rowmin", [BPC * PCH, NP], F32, kind="ExternalOutput").ap()
    colmin = nc.dram_tensor("colmin", [BPC, M], F32, kind="ExternalOutput").ap()

    with tile.TileContext(nc) as tc, ExitStack() as ctx:
        const_pool = ctx.enter_context(tc.tile_pool(name="const", bufs=1))
        neg_t = const_pool.tile([PCH, FDV], F32)
        nc.gpsimd.memset(neg_t[:], -BIG)

        lr_pool = ctx.enter_context(tc.tile_pool(name="lr", bufs=2))
        col_pool = ctx.enter_context(tc.tile_pool(name="col", bufs=NG + 1))
        red_pool = ctx.enter_context(tc.tile_pool(name="red", bufs=2))
        d2_pool = ctx.enter_context(tc.tile_pool(name="d2", bufs=4))
        acc_pool = ctx.enter_context(tc.tile_pool(name="acc", bufs=2))
        scr_pool = ctx.enter_context(tc.tile_pool(name="scr", bufs=8))
        psum_pool = ctx.enter_context(tc.tile_pool(name="psum", bufs=2, space="PSUM"))

        for i in range(BPC):
            L = lr_pool.tile([K, N], BF16, tag="L")
            nc.sync.dma_start(L[:], lhs[K * i : K * (i + 1), :])
            R = lr_pool.tile([K, M], BF16, tag="R")
            nc.sync.dma_start(R[:], rhs[K * i : K * (i + 1), :])

            colstate = [
                col_pool.tile([PCH, FDV], F32, tag="cs", name=f"cs_{i}_{g}")
                for g in range(NG)
            ]
            rowacc = acc_pool.tile([PCH, NP], F32, tag="rowacc")

            rowpart = scr_pool.tile(
                [PCH, NP * NG], F32, tag="rowpart", name=f"rp_{i}"
            )
            for p in range(NP):
                for g in range(NG):
                    # stage a [PCH, FDV] block of s = -d2 into SBUF via
                    # ScalarE so both DVE passes below run from SBUF
                    d2 = d2_pool.tile([PCH, FDV], F32, tag="d2")
                    for half in range(FDV // FD):
                        ps = psum_pool.tile([PCH, FD], F32, tag="ps")
                        base = FDV * g + FD * half
                        for s in range(FD // MMN):
                            nc.tensor.matmul(
                                ps[:, MMN * s : MMN * (s + 1)],
                                lhsT=L[:, PCH * p : PCH * (p + 1)],
                                rhs=R[:, base + MMN * s : base + MMN * (s + 1)],
                                start=True,
                                stop=True,
                            )
                        nc.scalar.copy(d2[:, FD * half : FD * (half + 1)], ps[:])
                    # row (pred-point) partial max over this gt chunk
                    nc.vector.tensor_reduce(
                        out=rowpart[:, p * NG + g : p * NG + g + 1],
                        in_=d2[:],
                        axis=mybir.AxisListType.X,
                        op=MAX,
                    )
                    # colstate accumulation (max over pred chunks).
                    src0 = neg_t[:] if p == 0 else colstate[g][:]
                    nc.vector.tensor_tensor(
                        out=colstate[g][:], in0=src0, in1=d2[:], op=MAX
                    )
            # fold all NP x NG partials into the final row maxes in one op
            nc.vector.tensor_reduce(
                out=rowacc[:],
                in_=rowpart[:].rearrange("p (a b) -> p a b", b=NG),
                axis=mybir.AxisListType.X,
                op=MAX,
            )

            # Column maxes: partition all-reduce on GpSimd, then ship row 0.
            for g in range(NG):
                csr = red_pool.tile([PCH, FDV], F32, tag="csr", name=f"csr_{i}_{g}")
                nc.gpsimd.partition_all_reduce(
                    csr[:], colstate[g][:], channels=PCH,
                    reduce_op=bass_isa.ReduceOp.max,
                )
                nc.sync.dma_start(
                    colmin[i : i + 1, FDV * g : FDV * (g + 1)], csr[0:1, :]
                )

            nc.sync.dma_start(rowmin[PCH * i : PCH * (i + 1), :], rowacc[:])

    nc.compile()
    return nc


@lru_cache(maxsize=1)
def _get_program():
    return _build_program()


def _split3(x):
    """fp32 -> three bf16 terms whose sum matches x to ~2^-27 rel."""
    h = x.astype(NPBF16)
    r = x - h.astype(np.float32)
    m = r.astype(NPBF16)
    l = (r - m.astype(np.float32)).astype(NPBF16)
    return h, m, l


def _make_inputs(pred, gt):
    """Host-side packing of the K=24 bf16 split operands (for -d2), per core."""
    pred = np.ascontiguousarray(pred, dtype=np.float32)
    gt = np.ascontiguousarray(gt, dtype=np.float32)
    p2 = np.einsum("bnd,bnd->bn", pred, pred)
    g2 = np.einsum("bmd,bmd->bm", gt, gt)
    Lr, Rr = [], []
    for d in range(3):
        u = np.float32(2.0) * pred[:, :, d]  # +2 so the dot yields -d2
        v = gt[:, :, d]
        uh, um, ul = _split3(u)
        vh, vm, vl = _split3(v)
        # product pairs kept: hh, hm, mh, hl, lh, mm
        Lr += [uh, uh, um, uh, ul, um]
        Rr += [vh, vm, vh, vl, vh, vm]
    ph, pm, pl = _split3(-p2)
    gh, gm, gl = _split3(g2)
    ones_n = np.ones_like(p2, dtype=NPBF16)
    neg_n = -ones_n
    ones_m = np.ones_like(g2, dtype=NPBF16)
    Lr += [ph, pm, pl, neg_n, neg_n, neg_n]
    Rr += [ones_m, ones_m, ones_m, gh, gm, gl]
    lhs = np.stack(Lr, axis=1)  # [B, K, N] bf16
    rhs = np.stack(Rr, axis=1)  # [B, K, M] bf16
    in_maps = []
    for c in range(NCORES):
        sl = slice(c * BPC, (c + 1) * BPC)
        in_maps.append(
            {
                "lhs": np.ascontiguousarray(lhs[sl].reshape(BPC * K, N)),
                "rhs": np.ascontiguousarray(rhs[sl].reshape(BPC * K, M)),
            }
        )
    return in_maps


def _finish(results):
    # device values are maxes of -d2: negate back to d2 mins
    rowmins = -np.stack([r["rowmin"] for r in results])  # [8, BPC*128, 32]
    colmins = -np.stack([r["colmin"] for r in results])  # [8, BPC, M]
    ch2 = np.sqrt(np.maximum(rowmins.astype(np.float64), 1e-12)).mean()
    ch1 = np.sqrt(np.maximum(colmins.astype(np.float64), 1e-12)).mean()
    return np.asarray(ch1 + ch2, dtype=np.float32)


def kernel(pred, gt):
    nc = _get_program()
    in_maps = _make_inputs(pred, gt)
    res = run_bass_kernel_spmd(nc, in_maps, list(range(NCORES)))
    return _finish(res.results)


if __name__ == "__main__":
    rng = np.random.default_rng(0)
    pred = rng.standard_normal((B, N, 3), dtype=np.float32)
    gt = rng.standard_normal((B, M, 3), dtype=np.float32)
    print(kernel(pred, gt))


# revision 31
# speedup vs baseline: 1.2121x; 1.1752x over previous
"""Chamfer loss kernel for Trainium2 (8 NeuronCores, SPMD data-parallel over batch).

Math: the device computes s = -d2 where d2[n, m] = |p_n|^2 + |g_m|^2 - 2 p_n.g_m,
so every min the loss needs becomes a max on device (GpSimd's
partition_all_reduce only offers max). s is produced directly by an augmented
matmul on the PE. To run the PE at bf16 rate (1 cycle/row instead of fp32's 4)
without losing fp32 accuracy, every fp32 operand is split into three bf16
terms (h + m + l); retaining the product pairs hh, hm, mh, hl, lh, mm
reproduces each fp32 product to ~2^-27 rel. With 3 coords x 6 pairs + 3 |p|^2
rows + 3 |g|^2 rows the contraction dim is K=24, all bf16, accumulated
exactly into fp32 PSUM.

Per PSUM megatile [128, 1024] (2 banks): ScalarE copies s to SBUF (freeing
the PSUM bank early and keeping both DVE reads off PSUM); DVE takes the
free-axis row max and the running column-state max; GpSimd finishes the
column maxes with partition_all_reduce(max). All DVE fp32 ops run at 1x
(~1.1 elem/cycle/lane measured) — two passes over every element is the hard
floor, and DVE sits at ~97% busy. sqrt + means run on the host (min/max
commute with sqrt/clamp after negation).

Each core handles 4 of the 32 batches. No collectives; host combines scalars.
"""

import sys

for _p in ("/opt/trn_rl_repo",):
    if _p not in sys.path:
        sys.path.insert(0, _p)

from contextlib import ExitStack
from functools import lru_cache

import ml_dtypes
import numpy as np

import concourse.bass as bass
import concourse.tile as tile
from concourse import bacc, bass_isa, mybir
from concourse.bass_utils import run_bass_kernel_spmd

F32 = mybir.dt.float32
BF16 = mybir.dt.bfloat16
MAX = mybir.AluOpType.max
NPBF16 = ml_dtypes.bfloat16

B, N, M = 32, 4096, 4096
NCORES = 8
BPC = B // NCORES  # batches per core
K = 24             # augmented contraction dim (3 coords x 6 bf16 pairs + 2x3 norm rows)
PCH = 128          # pred chunk size (PE partitions)
NP = N // PCH      # 32 pred chunks
FD = 2048          # psum tile free size (4 PSUM banks)
FDV = 2048         # DVE working chunk (psum tiles staged into SBUF by ScalarE)
NG = M // FDV      # gt chunks per batch row pass
MMN = 512          # matmul moving free dim (one fp32 PSUM bank)
BIG = 3.0e38
# p-chunks whose column pass runs as a GpSimd partition_all_reduce instead of
# the DVE colstate tensor_tensor: odd p below 30 (15 of 32) balances the two
# engines (DVE ~2.27us/chunk vs GpSimd ~7.5us/chunk).
GP_SET = frozenset(p for p in range(NP) if p % 2 == 1 and p < 30)
NGP = len(GP_SET)


def _build_program():
    nc = bacc.Bacc(
        "TRN2", target_bir_lowering=False, debug=False, num_devices=NCORES
    )
    lhs = nc.dram_tensor("lhs", [BPC * K, N], BF16, kind="ExternalInput").ap()
    rhs = nc.dram_tensor("rhs", [BPC * K, M], BF16, kind="ExternalInput").ap()
    rowmin = nc.dram_tensor("rowmin", [BPC * PCH, NP], F32, kind="ExternalOutput").ap()
    # per (batch, g-chunk): row 0 = colstate all-reduce over the DVE-handled
    # p-chunks; rows 1..NGP = GpSimd per-chunk column partials. Host combines.
    colmin = nc.dram_tensor(
        "colmin", [BPC * NG * (NGP + 1), FDV], F32, kind="ExternalOutput"
    ).ap()

    with tile.TileContext(nc) as tc, ExitStack() as ctx:
        const_pool = ctx.enter_context(tc.tile_pool(name="const", bufs=1))
        neg_t = const_pool.tile([PCH, FDV], F32)
        nc.gpsimd.memset(neg_t[:], -BIG)

        lr_pool = ctx.enter_context(tc.tile_pool(name="lr", bufs=2))
        col_pool = ctx.enter_context(tc.tile_pool(name="col", bufs=NG + 1))
        red_pool = ctx.enter_context(tc.tile_pool(name="red", bufs=4))
        d2_pool = ctx.enter_context(tc.tile_pool(name="d2", bufs=4))
        acc_pool = ctx.enter_context(tc.tile_pool(name="acc", bufs=2))
        scr_pool = ctx.enter_context(tc.tile_pool(name="scr", bufs=8))
        psum_pool = ctx.enter_context(tc.tile_pool(name="psum", bufs=2, space="PSUM"))

        for i in range(BPC):
            L = lr_pool.tile([K, N], BF16, tag="L")
            nc.sync.dma_start(L[:], lhs[K * i : K * (i + 1), :])
            R = lr_pool.tile([K, M], BF16, tag="R")
            nc.sync.dma_start(R[:], rhs[K * i : K * (i + 1), :])

            colstate = [
                col_pool.tile([PCH, FDV], F32, tag="cs", name=f"cs_{i}_{g}")
                for g in range(NG)
            ]
            rowacc = acc_pool.tile([PCH, NP], F32, tag="rowacc")

            rowpart = scr_pool.tile(
                [PCH, NP * NG], F32, tag="rowpart", name=f"rp_{i}"
            )
            for p in range(NP):
                for g in range(NG):
                    # stage a [PCH, FDV] block of s = -d2 into SBUF via
                    # ScalarE so both DVE passes below run from SBUF
                    d2 = d2_pool.tile([PCH, FDV], F32, tag="d2")
                    for half in range(FDV // FD):
                        ps = psum_pool.tile([PCH, FD], F32, tag="ps")
                        base = FDV * g + FD * half
                        for s in range(FD // MMN):
                            nc.tensor.matmul(
                                ps[:, MMN * s : MMN * (s + 1)],
                                lhsT=L[:, PCH * p : PCH * (p + 1)],
                                rhs=R[:, base + MMN * s : base + MMN * (s + 1)],
                                start=True,
                                stop=True,
                            )
                        nc.scalar.copy(d2[:, FD * half : FD * (half + 1)], ps[:])
                    # row (pred-point) partial max over this gt chunk
                    nc.vector.tensor_reduce(
                        out=rowpart[:, p * NG + g : p * NG + g + 1],
                        in_=d2[:],
                        axis=mybir.AxisListType.X,
                        op=MAX,
                    )
                    if p in GP_SET:
                        # column partial for this chunk alone on GpSimd;
                        # shipped to DRAM, host min-combines
                        csr = red_pool.tile(
                            [PCH, FDV], F32, tag="csr", name=f"gp_{i}_{p}_{g}"
                        )
                        nc.gpsimd.partition_all_reduce(
                            csr[:], d2[:], channels=PCH,
                            reduce_op=bass_isa.ReduceOp.max,
                        )
                        row = (i * NG + g) * (NGP + 1) + 1 + (p - 1) // 2
                        nc.sync.dma_start(colmin[row : row + 1, :], csr[0:1, :])
                    else:
                        # colstate accumulation (max over DVE-handled chunks)
                        src0 = neg_t[:] if p == 0 else colstate[g][:]
                        nc.vector.tensor_tensor(
                            out=colstate[g][:], in0=src0, in1=d2[:], op=MAX
                        )
            # fold all NP x NG partials into the final row maxes in one op
            nc.vector.tensor_reduce(
                out=rowacc[:],
                in_=rowpart[:].rearrange("p (a b) -> p a b", b=NG),
                axis=mybir.AxisListType.X,
                op=MAX,
            )

            # Column maxes of the DVE-accumulated state: partition all-reduce
            # on GpSimd, shipped as row 0 of each (batch, g) group.
            for g in range(NG):
                csr = red_pool.tile([PCH, FDV], F32, tag="csr", name=f"csr_{i}_{g}")
                nc.gpsimd.partition_all_reduce(
                    csr[:], colstate[g][:], channels=PCH,
                    reduce_op=bass_isa.ReduceOp.max,
                )
                row = (i * NG + g) * (NGP + 1)
                nc.sync.dma_start(colmin[row : row + 1, :], csr[0:1, :])

            nc.sync.dma_start(rowmin[PCH * i : PCH * (i + 1), :], rowacc[:])

    nc.compile()
    return nc


@lru_cache(maxsize=1)
def _get_program():
    return _build_program()


def _split3(x):
    """fp32 -> three bf16 terms whose sum matches x to ~2^-27 rel."""
    h = x.astype(NPBF16)
    r = x - h.astype(np.float32)
    m = r.astype(NPBF16)
    l = (r - m.astype(np.float32)).astype(NPBF16)
    return h, m, l


def _make_inputs(pred, gt):
    """Host-side packing of the K=24 bf16 split operands (for -d2), per core."""
    pred = np.ascontiguousarray(pred, dtype=np.float32)
    gt = np.ascontiguousarray(gt, dtype=np.float32)
    p2 = np.einsum("bnd,bnd->bn", pred, pred)
    g2 = np.einsum("bmd,bmd->bm", gt, gt)
    Lr, Rr = [], []
    for d in range(3):
        u = np.float32(2.0) * pred[:, :, d]  # +2 so the dot yields -d2
        v = gt[:, :, d]
        uh, um, ul = _split3(u)
        vh, vm, vl = _split3(v)
        # product pairs kept: hh, hm, mh, hl, lh, mm
        Lr += [uh, uh, um, uh, ul, um]
        Rr += [vh, vm, vh, vl, vh, vm]
    ph, pm, pl = _split3(-p2)
    gh, gm, gl = _split3(g2)
    ones_n = np.ones_like(p2, dtype=NPBF16)
    neg_n = -ones_n
    ones_m = np.ones_like(g2, dtype=NPBF16)
    Lr += [ph, pm, pl, neg_n, neg_n, neg_n]
    Rr += [ones_m, ones_m, ones_m, gh, gm, gl]
    lhs = np.stack(Lr, axis=1)  # [B, K, N] bf16
    rhs = np.stack(Rr, axis=1)  # [B, K, M] bf16
    in_maps = []
    for c in range(NCORES):
        sl = slice(c * BPC, (c + 1) * BPC)
        in_maps.append(
            {
                "lhs": np.ascontiguousarray(lhs[sl].reshape(BPC * K, N)),
                "rhs": np.ascontiguousarray(rhs[sl].reshape(BPC * K, M)),
            }
        )
    return in_maps


def _finish(results):
    # device values are maxes of -d2: negate back to d2 mins
    rowmins = -np.stack([r["rowmin"] for r in results])  # [8, BPC*128, 32]
    colraw = np.stack([r["colmin"] for r in results])  # [8, BPC*NG*(NGP+1), FDV]
    colmins = -(
        colraw.reshape(NCORES, BPC, NG, NGP + 1, FDV).max(axis=3)
    ).reshape(NCORES, BPC, M)
    ch2 = np.sqrt(np.maximum(rowmins.astype(np.float64), 1e-12)).mean()
    ch1 = np.sqrt(np.maximum(colmins.astype(np.float64), 1e-12)).mean()
    return np.asarray(ch1 + ch2, dtype=np.float32)


def kernel(pred, gt):
    nc = _get_program()
    in_maps = _make_inputs(pred, gt)
    res = run_bass_kernel_spmd(nc, in_maps, list(range(NCORES)))
    return _finish(res.results)


if __name__ == "__main__":
    rng = np.random.default_rng(0)
    pred = rng.standard_normal((B, N, 3), dtype=np.float32)
    gt = rng.standard_normal((B, M, 3), dtype=np.float32)
    print(kernel(pred, gt))
